# revision 1
# baseline (speedup 1.0000x reference)
"""ContinuousFilterConvolution (gnn message passing) on 8 Trainium2 cores.

Strategy (edge/dest data-parallel, no collectives):
  - Sort edges by dest; group dest nodes into 128-row blocks (392 blocks
    padded), 49 blocks per core. Each core owns disjoint output rows.
  - Host precomputes per-edge RBF features (function of geometry only) and
    index tables; device does all node_feats gathers, the 2-layer MLP
    (bf16 matmuls, f32 PSUM), the gather-multiply, and the segment-sum
    (one-hot matmul accumulated in PSUM per dest block).
  - node_feats gathers use the SWDGE dma_gather custom instruction
    (int16 indices -> the node table is addressed as lo/hi halves).
"""
import sys
sys.path.insert(0, "/opt/trn_rl_repo")
import numpy as np
import ml_dtypes

import concourse.bass as bass
import concourse.mybir as mybir
import concourse.tile as tile
from concourse import bacc
from concourse.bass_utils import run_bass_kernel_spmd

bf16 = ml_dtypes.bfloat16
f32 = np.float32
dt = mybir.dt

P = 128
V = 50_000
E = 1_600_000
DH = 128
NB = 16
D_MIN, D_MAX = 0.0, 4.5
N_CORES = 8
HALF = 32_768          # int16 index range split of the node table
GB_TILES = 8           # tiles per dma_gather call (1024 rows, HW ring limit)

NBLK = -(-V // P)                          # 391
NBLK_PAD = -(-NBLK // N_CORES) * N_CORES   # 392
NBPC = NBLK_PAD // N_CORES                 # 49


def kernel(**inputs):
    node_feats = np.asarray(inputs["node_feats"], dtype=f32)
    coords = np.asarray(inputs["coords"], dtype=f32)
    src = np.asarray(inputs["src"])
    dest = np.asarray(inputs["dest"])
    W1 = np.asarray(inputs["W1"], dtype=f32)
    W2 = np.asarray(inputs["W2"], dtype=f32)

    out, _ = _run(node_feats, coords, src, dest, W1, W2)
    return out


def _run(node_feats, coords, src, dest, W1, W2, want_runner=False):
    # One shared SPMD program means the per-block lo/hi gather-table choice
    # must be identical across cores; the host pads each block position's
    # lo section to the max over cores, rounded to GB_TILES tiles.
    cores, t_fix, cap, shared_lo = _host_prep_shared(
        node_feats, coords, src, dest)
    nt_core = NBPC * t_fix

    nc = bacc.Bacc("TRN2", target_bir_lowering=False, debug=False,
                   enable_asserts=False, num_devices=N_CORES)
    nf_d = nc.dram_tensor("node_feats", [V, DH], dt.float32,
                          kind="ExternalInput").ap()
    idx_d = nc.dram_tensor("idx", [P, nt_core * P // 16], dt.int16,
                           kind="ExternalInput").ap()
    dest_d = nc.dram_tensor("dest_t", [P, nt_core], dt.float32,
                            kind="ExternalInput").ap()
    rbf_d = nc.dram_tensor("rbf_t", [NB, nt_core * P], dt.bfloat16,
                           kind="ExternalInput").ap()
    iota_d = nc.dram_tensor("iota", [P, P], dt.bfloat16,
                            kind="ExternalInput").ap()
    w1_d = nc.dram_tensor("w1", [NB, DH], dt.bfloat16,
                          kind="ExternalInput").ap()
    w2_d = nc.dram_tensor("w2", [DH, DH], dt.bfloat16,
                          kind="ExternalInput").ap()
    out_d = nc.dram_tensor("out", [NBPC * P, DH], dt.float32,
                           kind="ExternalOutput").ap()
    nf_lo = nf_d[:HALF, :]
    nf_hi = nf_d[HALF:, :]

    Relu = mybir.ActivationFunctionType.Relu
    with tile.TileContext(nc) as tc:
        with (
            tc.tile_pool(name="const", bufs=1) as cpool,
            tc.tile_pool(name="io", bufs=2) as iopool,
            tc.tile_pool(name="gather", bufs=2) as gpool,
            tc.tile_pool(name="work", bufs=3) as wpool,
            tc.tile_pool(name="spool", bufs=4) as spool,
            tc.tile_pool(name="psum", bufs=2, space="PSUM") as ppool,
            tc.tile_pool(name="acc", bufs=2, space="PSUM") as apool,
        ):
            iota_sb = cpool.tile([P, P], dt.bfloat16)
            nc.sync.dma_start(iota_sb[:], iota_d[:])
            w1_sb = cpool.tile([NB, DH], dt.bfloat16)
            nc.sync.dma_start(w1_sb[:], w1_d[:])
            w2_sb = cpool.tile([DH, DH], dt.bfloat16)
            nc.sync.dma_start(w2_sb[:], w2_d[:])
            idx_sb = cpool.tile([P, nt_core * P // 16], dt.int16)
            nc.sync.dma_start(idx_sb[:], idx_d[:])
            dest_sb = cpool.tile([P, nt_core], dt.float32)
            nc.sync.dma_start(dest_sb[:], dest_d[:])

            for b in range(NBPC):
                t0 = b * t_fix
                rbf_sb = iopool.tile([NB, cap], dt.bfloat16, tag="rbf")
                nc.sync.dma_start(rbf_sb[:], rbf_d[:, t0 * P:(t0 + t_fix) * P])
                nf_sb = gpool.tile([P, cap], dt.float32, tag="nf")
                nf3 = nf_sb[:].rearrange("p (c e) -> p c e", e=DH)
                # gather runs of GB_TILES chunks; table per run from the
                # shared lo/hi boundary (a multiple of GB_TILES)
                for c0 in range(0, t_fix, GB_TILES):
                    nch = min(GB_TILES, t_fix - c0)
                    n_rows = nch * P
                    table = nf_lo if c0 < shared_lo[b] else nf_hi
                    nc.gpsimd.dma_gather(
                        out_ap=nf3[:, c0:c0 + nch, :],
                        in_ap=table,
                        idxs_ap=idx_sb[:, (t0 * P + c0 * P) // 16:
                                       (t0 * P + c0 * P + n_rows) // 16],
                        num_idxs=n_rows, num_idxs_reg=n_rows,
                        elem_size=DH, elem_step=DH)
                acc = apool.tile([P, DH], dt.float32, tag="acc")
                for g0 in range(0, t_fix, 4):
                    gn = min(4, t_fix - g0)
                    W = gn * DH
                    m1 = ppool.tile([DH, 512], dt.float32, tag="m1")
                    nc.tensor.matmul(m1[:, :W], lhsT=w1_sb[:],
                                     rhs=rbf_sb[:, g0 * P:g0 * P + W],
                                     start=True, stop=True)
                    s1 = wpool.tile([DH, 512], dt.bfloat16, tag="s1")
                    nc.scalar.activation(s1[:, :W], m1[:, :W], Relu)
                    m2 = ppool.tile([P, 512], dt.float32, tag="m2")
                    for j in range(gn):
                        nc.tensor.matmul(m2[:, j * DH:(j + 1) * DH],
                                         lhsT=s1[:, j * DH:(j + 1) * DH],
                                         rhs=w2_sb[:], start=True, stop=True)
                    s2 = wpool.tile([P, 512], dt.bfloat16, tag="s2")
                    nc.scalar.activation(s2[:, :W], m2[:, :W], Relu)
                    msg = wpool.tile([P, 512], dt.bfloat16, tag="msg")
                    nc.vector.tensor_tensor(
                        out=msg[:, :W], in0=s2[:, :W],
                        in1=nf_sb[:, g0 * DH:g0 * DH + W],
                        op=mybir.AluOpType.mult)
                    for j in range(gn):
                        t = g0 + j
                        S = spool.tile([P, P], dt.bfloat16, tag="S")
                        nc.vector.tensor_scalar(
                            out=S[:], in0=iota_sb[:],
                            scalar1=dest_sb[:, t0 + t:t0 + t + 1],
                            scalar2=None, op0=mybir.AluOpType.is_equal)
                        nc.tensor.matmul(acc[:], lhsT=S[:],
                                         rhs=msg[:, j * DH:(j + 1) * DH],
                                         start=(t == 0), stop=(t == t_fix - 1))
                outsb = wpool.tile([P, DH], dt.float32, tag="out")
                nc.vector.tensor_copy(out=outsb[:], in_=acc[:])
                nc.sync.dma_start(out_d[b * P:(b + 1) * P, :], outsb[:])
    nc.finalize()

    iota_np = np.tile(np.arange(P, dtype=f32), (P, 1)).astype(bf16)
    in_maps = []
    for c in range(N_CORES):
        in_maps.append({
            "node_feats": node_feats,
            "idx": cores[c]["idx"],
            "dest_t": cores[c]["dest_t"],
            "rbf_t": cores[c]["rbf_t"],
            "iota": iota_np,
            "w1": W1.astype(bf16),
            "w2": W2.astype(bf16),
        })
    res = run_bass_kernel_spmd(nc, in_maps, core_ids=list(range(N_CORES)))
    out_full = np.concatenate([res.results[c]["out"] for c in range(N_CORES)],
                              axis=0)[:V]
    if want_runner:
        return out_full.astype(f32), (nc, in_maps)
    return out_full.astype(f32), None


def _host_prep_shared(node_feats, coords, src, dest):
    """Like _host_prep but the per-block lo/hi boundary (in chunks of
    GB_TILES tiles) is shared across cores: per global block b the boundary
    is max over cores of that core's block-b lo chunk count, rounded up to
    a GB_TILES multiple. Edges are placed so lo edges live strictly below
    the boundary and hi edges strictly above; fill slots below boundary
    use idx 0 (lo table), above use idx 0 (hi table) - both with rbf=0.
    Returns (cores, t_fix, cap, shared_lo[NBPC] in chunk units)."""
    order = np.argsort(dest, kind="stable")
    src_s = src[order].astype(np.int64)
    dest_s = dest[order].astype(np.int64)
    blk = dest_s >> 7
    order2 = np.lexsort((src_s, blk))
    src_s = src_s[order2]
    dest_s = dest_s[order2]
    blk = blk[order2]

    cnt = np.bincount(blk, minlength=NBLK_PAD)
    is_hi = src_s >= HALF
    n_lo = np.bincount(blk[~is_hi], minlength=NBLK_PAD)
    n_hi = cnt - n_lo

    # shared lo boundary per block-position (0..NBPC): max over cores,
    # in GB_TILES*P row units
    GBR = GB_TILES * P
    n_lo_by_pos = n_lo.reshape(N_CORES, NBPC)
    n_hi_by_pos = n_hi.reshape(N_CORES, NBPC)
    lo_cap_pos = -(-n_lo_by_pos.max(0) // GBR) * GBR        # [NBPC] rows
    # capacity: lo_cap + hi must fit; t_fix covers worst block
    need = lo_cap_pos[None, :] + n_hi_by_pos
    t_fix = int(-(-int(need.max()) // P))
    cap = t_fix * P

    mu = np.linspace(D_MIN, D_MAX, NB, dtype=f32)
    width = (D_MAX - D_MIN) / (NB - 1)
    coeff = -0.5 / (width * width)
    diff = coords[src_s] - coords[dest_s]
    d = np.sqrt((diff * diff).sum(-1).astype(f32))
    rbf = np.exp(coeff * np.square(d[:, None] - mu)).astype(f32)

    lo_cap_full = np.tile(lo_cap_pos, N_CORES)              # [NBLK_PAD] rows
    block_start = np.zeros(NBLK_PAD + 1, np.int64)
    np.cumsum(cnt, out=block_start[1:])
    idx_in_block = np.arange(len(src_s), dtype=np.int64) - block_start[blk]
    rank_hi = idx_in_block - n_lo[blk]
    pos = blk * cap + np.where(is_hi, lo_cap_full[blk] + rank_hi, idx_in_block)

    epad = NBLK_PAD * cap
    idx16 = np.zeros(epad, np.int16)
    destrel = np.full(epad, 200.0, f32)
    rbf_p = np.zeros((epad, NB), f32)
    idx16[pos] = np.where(is_hi, src_s - HALF, src_s).astype(np.int16)
    destrel[pos] = (dest_s & 127).astype(f32)
    rbf_p[pos] = rbf

    nt_core = NBPC * t_fix
    rows_core = nt_core * P
    cores = []
    for c in range(N_CORES):
        sl = slice(c * rows_core, (c + 1) * rows_core)
        idx_c = idx16[sl]
        wrapped = np.tile(
            np.ascontiguousarray(idx_c.reshape(rows_core // 16, 16).T),
            (8, 1))
        dest_t = np.ascontiguousarray(destrel[sl].reshape(nt_core, P).T)
        rbf_t = np.ascontiguousarray(rbf_p[sl].T.astype(bf16))
        cores.append({"idx": wrapped, "dest_t": dest_t, "rbf_t": rbf_t})

    shared_lo_chunks = (lo_cap_pos // P).astype(np.int64)   # in tile units
    return cores, t_fix, cap, shared_lo_chunks



# revision 8
# speedup vs baseline: 1.5180x; 1.5180x over previous
"""ContinuousFilterConvolution (gnn message passing) on 8 Trainium2 cores.

Strategy (edge/dest data-parallel, no collectives):
  - Sort edges by dest; group dest nodes into 128-row blocks (392 blocks
    padded), 49 blocks per core. Each core owns disjoint output rows.
  - Host precomputes per-edge RBF features (function of geometry only) and
    index tables; device does all node_feats gathers, the 2-layer MLP
    (bf16 matmuls, f32 PSUM), the gather-multiply, and the segment-sum
    (one-hot matmul accumulated in PSUM per dest block).
  - node_feats gathers use the SWDGE dma_gather custom instruction
    (int16 indices -> the node table is addressed as lo/hi halves).
    The table is bf16 (halves gather bytes) and consecutive gather calls
    rotate over 4 SWDGE queues: a single queue is HBM-latency-bound at
    ~26 GB/s/core; 4 queues overlap to ~97 GB/s/core.
"""
import sys
sys.path.insert(0, "/opt/trn_rl_repo")
import numpy as np
import ml_dtypes

import concourse.bass as bass
import concourse.mybir as mybir
import concourse.tile as tile
from concourse import bacc
from concourse.bass_utils import run_bass_kernel_spmd

bf16 = ml_dtypes.bfloat16
f32 = np.float32
dt = mybir.dt

P = 128
V = 50_000
E = 1_600_000
DH = 128
NB = 16
D_MIN, D_MAX = 0.0, 4.5
N_CORES = 8
HALF = 32_768          # int16 index range split of the node table
GB_TILES = 8           # tiles per dma_gather call (1024 rows, HW ring limit)

NBLK = -(-V // P)                          # 391
NBLK_PAD = -(-NBLK // N_CORES) * N_CORES   # 392
NBPC = NBLK_PAD // N_CORES                 # 49


def kernel(**inputs):
    node_feats = np.asarray(inputs["node_feats"], dtype=f32)
    coords = np.asarray(inputs["coords"], dtype=f32)
    src = np.asarray(inputs["src"])
    dest = np.asarray(inputs["dest"])
    W1 = np.asarray(inputs["W1"], dtype=f32)
    W2 = np.asarray(inputs["W2"], dtype=f32)

    out, _ = _run(node_feats, coords, src, dest, W1, W2)
    return out


def _run(node_feats, coords, src, dest, W1, W2, want_runner=False):
    # One shared SPMD program means the per-block lo/hi gather-table choice
    # must be identical across cores; the host pads each block position's
    # lo section to the max over cores, rounded to GB_TILES tiles.
    cores, t_fix, cap, shared_lo = _host_prep_shared(
        node_feats, coords, src, dest)
    nc = _build(t_fix, shared_lo)

    iota_np = np.tile(np.arange(P, dtype=f32), (P, 1)).astype(bf16)
    nfb = np.ascontiguousarray(node_feats.astype(bf16))
    in_maps = []
    for c in range(N_CORES):
        in_maps.append({
            "node_feats": nfb,
            "idx": cores[c]["idx"],
            "dest_t": cores[c]["dest_t"],
            "rbf_t": cores[c]["rbf_t"],
            "iota": iota_np,
            "w1": W1.astype(bf16),
            "w2": W2.astype(bf16),
        })
    res = run_bass_kernel_spmd(nc, in_maps, core_ids=list(range(N_CORES)))
    out_full = np.concatenate([res.results[c]["out"] for c in range(N_CORES)],
                              axis=0)[:V]
    if want_runner:
        return out_full.astype(f32), (nc, in_maps)
    return out_full.astype(f32), None


def _build(t_fix, shared_lo):
    nt_core = NBPC * t_fix
    cap = t_fix * P

    nc = bacc.Bacc("TRN2", target_bir_lowering=False, debug=False,
                   enable_asserts=False, num_devices=N_CORES,
                   num_swdge_queues=4)
    nf_d = nc.dram_tensor("node_feats", [V, DH], dt.bfloat16,
                          kind="ExternalInput").ap()
    idx_d = nc.dram_tensor("idx", [P, nt_core * P // 16], dt.int16,
                           kind="ExternalInput").ap()
    dest_d = nc.dram_tensor("dest_t", [P, nt_core], dt.float32,
                            kind="ExternalInput").ap()
    rbf_d = nc.dram_tensor("rbf_t", [NB, nt_core * P], dt.bfloat16,
                           kind="ExternalInput").ap()
    iota_d = nc.dram_tensor("iota", [P, P], dt.bfloat16,
                            kind="ExternalInput").ap()
    w1_d = nc.dram_tensor("w1", [NB, DH], dt.bfloat16,
                          kind="ExternalInput").ap()
    w2_d = nc.dram_tensor("w2", [DH, DH], dt.bfloat16,
                          kind="ExternalInput").ap()
    out_d = nc.dram_tensor("out", [NBPC * P, DH], dt.float32,
                           kind="ExternalOutput").ap()
    nf_lo = nf_d[:HALF, :]
    nf_hi = nf_d[HALF:, :]

    Relu = mybir.ActivationFunctionType.Relu
    with tile.TileContext(nc) as tc:
        with (
            tc.tile_pool(name="const", bufs=1) as cpool,
            tc.tile_pool(name="io", bufs=2) as iopool,
            tc.tile_pool(name="gather", bufs=2) as gpool,
            tc.tile_pool(name="work", bufs=3) as wpool,
            tc.tile_pool(name="spool", bufs=4) as spool,
            tc.tile_pool(name="psum", bufs=2, space="PSUM") as ppool,
            tc.tile_pool(name="acc", bufs=2, space="PSUM") as apool,
        ):
            iota_sb = cpool.tile([P, P], dt.bfloat16)
            nc.sync.dma_start(iota_sb[:], iota_d[:])
            w1_sb = cpool.tile([NB, DH], dt.bfloat16)
            nc.sync.dma_start(w1_sb[:], w1_d[:])
            w2_sb = cpool.tile([DH, DH], dt.bfloat16)
            nc.sync.dma_start(w2_sb[:], w2_d[:])
            idx_sb = cpool.tile([P, nt_core * P // 16], dt.int16)
            nc.sync.dma_start(idx_sb[:], idx_d[:])
            dest_sb = cpool.tile([P, nt_core], dt.float32)
            nc.sync.dma_start(dest_sb[:], dest_d[:])

            qc = 0
            for b in range(NBPC):
                t0 = b * t_fix
                rbf_sb = iopool.tile([NB, cap], dt.bfloat16, tag="rbf")
                nc.sync.dma_start(rbf_sb[:], rbf_d[:, t0 * P:(t0 + t_fix) * P])
                nf_sb = gpool.tile([P, cap], dt.bfloat16, tag="nf")
                nf3 = nf_sb[:].rearrange("p (c e) -> p c e", e=DH)
                # gather runs of GB_TILES chunks; table per run from the
                # shared lo/hi boundary (a multiple of GB_TILES). Rotating
                # queue_num keeps 4 SWDGE rings draining concurrently (each
                # ring alone is HBM-latency-bound at ~26 GB/s).
                for c0 in range(0, t_fix, GB_TILES):
                    nch = min(GB_TILES, t_fix - c0)
                    n_rows = nch * P
                    table = nf_lo if c0 < shared_lo[b] else nf_hi
                    nc.gpsimd.dma_gather(
                        out_ap=nf3[:, c0:c0 + nch, :],
                        in_ap=table,
                        idxs_ap=idx_sb[:, (t0 * P + c0 * P) // 16:
                                       (t0 * P + c0 * P + n_rows) // 16],
                        num_idxs=n_rows, num_idxs_reg=n_rows,
                        elem_size=DH, elem_step=DH, queue_num=qc % 4)
                    qc += 1
                acc = apool.tile([P, DH], dt.float32, tag="acc")
                for g0 in range(0, t_fix, 4):
                    gn = min(4, t_fix - g0)
                    W = gn * DH
                    m1 = ppool.tile([DH, 512], dt.float32, tag="m1")
                    nc.tensor.matmul(m1[:, :W], lhsT=w1_sb[:],
                                     rhs=rbf_sb[:, g0 * P:g0 * P + W],
                                     start=True, stop=True)
                    s1 = wpool.tile([DH, 512], dt.bfloat16, tag="s1")
                    nc.scalar.activation(s1[:, :W], m1[:, :W], Relu)
                    m2 = ppool.tile([P, 512], dt.float32, tag="m2")
                    for j in range(gn):
                        nc.tensor.matmul(m2[:, j * DH:(j + 1) * DH],
                                         lhsT=s1[:, j * DH:(j + 1) * DH],
                                         rhs=w2_sb[:], start=True, stop=True)
                    s2 = wpool.tile([P, 512], dt.bfloat16, tag="s2")
                    nc.scalar.activation(s2[:, :W], m2[:, :W], Relu)
                    msg = wpool.tile([P, 512], dt.bfloat16, tag="msg")
                    nc.vector.tensor_tensor(
                        out=msg[:, :W], in0=s2[:, :W],
                        in1=nf_sb[:, g0 * DH:g0 * DH + W],
                        op=mybir.AluOpType.mult)
                    for j in range(gn):
                        t = g0 + j
                        S = spool.tile([P, P], dt.bfloat16, tag="S")
                        nc.vector.tensor_scalar(
                            out=S[:], in0=iota_sb[:],
                            scalar1=dest_sb[:, t0 + t:t0 + t + 1],
                            scalar2=None, op0=mybir.AluOpType.is_equal)
                        nc.tensor.matmul(acc[:], lhsT=S[:],
                                         rhs=msg[:, j * DH:(j + 1) * DH],
                                         start=(t == 0), stop=(t == t_fix - 1))
                outsb = wpool.tile([P, DH], dt.float32, tag="out")
                nc.vector.tensor_copy(out=outsb[:], in_=acc[:])
                nc.sync.dma_start(out_d[b * P:(b + 1) * P, :], outsb[:])
    nc.finalize()
    return nc


def _host_prep_shared(node_feats, coords, src, dest):
    """Like _host_prep but the per-block lo/hi boundary (in chunks of
    GB_TILES tiles) is shared across cores: per global block b the boundary
    is max over cores of that core's block-b lo chunk count, rounded up to
    a GB_TILES multiple. Edges are placed so lo edges live strictly below
    the boundary and hi edges strictly above; fill slots below boundary
    use idx 0 (lo table), above use idx 0 (hi table) - both with rbf=0.
    Returns (cores, t_fix, cap, shared_lo[NBPC] in chunk units)."""
    order = np.argsort(dest, kind="stable")
    src_s = src[order].astype(np.int64)
    dest_s = dest[order].astype(np.int64)
    blk = dest_s >> 7
    order2 = np.lexsort((src_s, blk))
    src_s = src_s[order2]
    dest_s = dest_s[order2]
    blk = blk[order2]

    cnt = np.bincount(blk, minlength=NBLK_PAD)
    is_hi = src_s >= HALF
    n_lo = np.bincount(blk[~is_hi], minlength=NBLK_PAD)
    n_hi = cnt - n_lo

    # shared lo boundary per block-position (0..NBPC): max over cores,
    # in GB_TILES*P row units
    GBR = GB_TILES * P
    n_lo_by_pos = n_lo.reshape(N_CORES, NBPC)
    n_hi_by_pos = n_hi.reshape(N_CORES, NBPC)
    lo_cap_pos = -(-n_lo_by_pos.max(0) // GBR) * GBR        # [NBPC] rows
    # capacity: lo_cap + hi must fit; t_fix covers worst block
    need = lo_cap_pos[None, :] + n_hi_by_pos
    t_fix = int(-(-int(need.max()) // P))
    cap = t_fix * P

    mu = np.linspace(D_MIN, D_MAX, NB, dtype=f32)
    width = (D_MAX - D_MIN) / (NB - 1)
    coeff = -0.5 / (width * width)
    diff = coords[src_s] - coords[dest_s]
    d = np.sqrt((diff * diff).sum(-1).astype(f32))
    rbf = np.exp(coeff * np.square(d[:, None] - mu)).astype(f32)

    lo_cap_full = np.tile(lo_cap_pos, N_CORES)              # [NBLK_PAD] rows
    block_start = np.zeros(NBLK_PAD + 1, np.int64)
    np.cumsum(cnt, out=block_start[1:])
    idx_in_block = np.arange(len(src_s), dtype=np.int64) - block_start[blk]
    rank_hi = idx_in_block - n_lo[blk]
    pos = blk * cap + np.where(is_hi, lo_cap_full[blk] + rank_hi, idx_in_block)

    epad = NBLK_PAD * cap
    idx16 = np.zeros(epad, np.int16)
    destrel = np.full(epad, 200.0, f32)
    rbf_p = np.zeros((epad, NB), f32)
    idx16[pos] = np.where(is_hi, src_s - HALF, src_s).astype(np.int16)
    destrel[pos] = (dest_s & 127).astype(f32)
    rbf_p[pos] = rbf

    nt_core = NBPC * t_fix
    rows_core = nt_core * P
    cores = []
    for c in range(N_CORES):
        sl = slice(c * rows_core, (c + 1) * rows_core)
        idx_c = idx16[sl]
        wrapped = np.tile(
            np.ascontiguousarray(idx_c.reshape(rows_core // 16, 16).T),
            (8, 1))
        dest_t = np.ascontiguousarray(destrel[sl].reshape(nt_core, P).T)
        rbf_t = np.ascontiguousarray(rbf_p[sl].T.astype(bf16))
        cores.append({"idx": wrapped, "dest_t": dest_t, "rbf_t": rbf_t})

    shared_lo_chunks = (lo_cap_pos // P).astype(np.int64)   # in tile units
    return cores, t_fix, cap, shared_lo_chunks

